# revision 11
# baseline (speedup 1.0000x reference)
import sys
if '/opt/trn_rl_repo' not in sys.path:
    sys.path.insert(0, '/opt/trn_rl_repo')
import numpy as np
import ml_dtypes

import concourse.bass as bass
import concourse.bacc as bacc
import concourse.mybir as mybir
import concourse.tile as tile
from concourse import library_config
from concourse.masks import make_identity
from concourse.bass_utils import run_bass_kernel_spmd
from concourse._compat import cdiv

NCORE = 8
N_NODES = 100000
NODE_DIM = 128
HID = 16
NGRAPH = 256
NCLS = 3
NLOC = 12544            # table slots per core
NREAL = 12500           # real nodes per core (100000/8)
NGRP = 1568             # slots per gpsimd group (NLOC/8)
NGP = 1664              # padded group width for pooling (13*128)
NCHUNK = 17             # dst chunks per group: 16 x 96 + 1 x 32
CHUNKS = [96] * 16 + [32]
CHUNK_OFF = [96 * i for i in range(16)] + [1536]
ZSLOT = NGRP - 1        # group-0 slot 1567 is always fake (zero row)
MODS = [("mri", 256), ("cog", 64), ("clin", 32), ("gen", 512)]

F32 = mybir.dt.float32
BF16 = mybir.dt.bfloat16
I16 = mybir.dt.int16
AX = mybir.AxisListType
OP = mybir.AluOpType
ACT = mybir.ActivationFunctionType


def _build(Ks, totI):
    """Per-core SPMD program. Ks[s][ch]: slots per dst for sub-block s,
    chunk ch (unified across cores/groups). totI: free width of the packed
    index param ([128, totI] int16)."""
    nc = bacc.Bacc(num_swdge_queues=1)
    P = {}
    P['x16'] = nc.declare_dram_parameter("x16", [NLOC, NODE_DIM], BF16, isOutput=False)
    P['W16'] = nc.declare_dram_parameter("W16", [NODE_DIM, HID], BF16, isOutput=False)
    P['dinvdst'] = nc.declare_dram_parameter("dinvdst", [128, NGP], F32, isOutput=False)
    P['brepT'] = nc.declare_dram_parameter("brepT", [128, 1], F32, isOutput=False)
    P['gidx2'] = nc.declare_dram_parameter("gidx2", [128, 8 * 13], F32, isOutput=False)
    P['giota'] = nc.declare_dram_parameter("giota", [128, NGRAPH], F32, isOutput=False)
    P['idxall'] = nc.declare_dram_parameter("idxall", [128, totI], I16, isOutput=False)
    for m, fdim in MODS:
        P[m + 'T'] = nc.declare_dram_parameter(m + 'T', [fdim, NGRAPH], F32, isOutput=False)
        P[m + 'W'] = nc.declare_dram_parameter(m + 'W', [fdim, 4], F32, isOutput=False)
        P[m + 'b'] = nc.declare_dram_parameter(m + 'b', [4, 1], F32, isOutput=False)
    P['cW1'] = nc.declare_dram_parameter("cW1", [32, HID], F32, isOutput=False)
    P['cb1'] = nc.declare_dram_parameter("cb1", [HID, 1], F32, isOutput=False)
    P['cW2'] = nc.declare_dram_parameter("cW2", [HID, NCLS], F32, isOutput=False)
    P['cb2'] = nc.declare_dram_parameter("cb2", [NCLS, 1], F32, isOutput=False)
    out = nc.declare_dram_parameter("out", [NGRAPH, NCLS], F32, isOutput=True)

    tloc = nc.dram_tensor("tloc", [16, NLOC], F32)
    table = nc.dram_tensor("table", [128, NLOC], F32, addr_space="Shared")
    poolloc = nc.dram_tensor("poolloc", [HID + 1, NGRAPH], F32)
    poolred = nc.dram_tensor("poolred", [HID + 1, NGRAPH], F32, addr_space="Shared")
    groups = [list(range(NCORE))]
    Jmax = max(CHUNKS[c] * Ks[s][c] for s in range(NCORE) for c in range(NCHUNK))

    with tile.TileContext(nc) as tc:
        with tc.tile_pool(name="pers", bufs=1) as pp, \
             tc.tile_pool(name="sb", bufs=2) as sb, \
             tc.tile_pool(name="blk", bufs=2) as bb, \
             tc.tile_pool(name="gat", bufs=3) as gb, \
             tc.tile_pool(name="ps", bufs=2, space="PSUM") as ps, \
             tc.tile_pool(name="pool_ps", bufs=1, space="PSUM") as pps:
            nc.gpsimd.load_library(library_config.ap_gather)

            # ---------- phase 1: tlocT = (x @ W * dinv_src)^T, AllGather ----------
            xT = pp.tile([128, NLOC], BF16)
            nc.sync.dma_start_transpose(xT[:], P['x16'][:])
            Wt = pp.tile([NODE_DIM, HID], BF16)
            nc.sync.dma_start(out=Wt[:], in_=P['W16'][:])
            dinvdst = pp.tile([128, NGP], F32)
            nc.sync.dma_start(out=dinvdst[:], in_=P['dinvdst'][:])
            brepT = pp.tile([128, 1], F32)
            nc.sync.dma_start(out=brepT[:], in_=P['brepT'][:])
            gidx2 = pp.tile([128, 8 * 13], F32)
            nc.sync.dma_start(out=gidx2[:], in_=P['gidx2'][:])
            giota = pp.tile([128, NGRAPH], F32)
            nc.sync.dma_start(out=giota[:], in_=P['giota'][:])
            ident = pp.tile([128, 128], F32)
            make_identity(nc, ident[:])
            ones_t = pp.tile([128, 1], F32)
            nc.vector.memset(ones_t[:], 1.0)

            for t in range(25):
                c0 = t * 512
                c1 = min(NLOC, c0 + 512)
                xwp = ps.tile([16, 512], F32, tag="p1ps")
                nc.tensor.matmul(xwp[:, :c1 - c0], Wt[:], xT[:, c0:c1],
                                 start=True, stop=True)
                tlocc = sb.tile([16, 512], F32, tag="tlocc")
                nc.vector.tensor_copy(tlocc[:, :c1 - c0], xwp[:, :c1 - c0])
                nc.sync.dma_start(out=tloc[:, c0:c1], in_=tlocc[:, :c1 - c0])
            nc.gpsimd.collective_compute(
                "AllGather", OP.bypass, replica_groups=groups,
                ins=[tloc[:]], outs=[table[:]])

            # ---------- phase 2: per sub-block SBUF gather + reduce ----------
            accum = pp.tile([128, NGP], F32)
            nc.vector.memset(accum[:], 0.0)

            ioff = 0
            for s in range(NCORE):
                blk = bb.tile([128, NLOC], F32, tag="blk")
                for g in range(8):
                    nc.sync.dma_start(out=blk[16 * g:16 * g + 16, :],
                                      in_=table[16 * s:16 * s + 16, :])
                for ch in range(NCHUNK):
                    n = CHUNKS[ch]
                    K = Ks[s][ch]
                    J = n * K
                    it = gb.tile([128, cdiv(Jmax, 16)], I16, tag="it")
                    nc.sync.dma_start(out=it[:, :J // 16],
                                      in_=P['idxall'][:, ioff:ioff + J // 16])
                    ioff += J // 16
                    st = gb.tile([128, Jmax], F32, tag="st")
                    nc.gpsimd.ap_gather(
                        st[:, :J], blk[:], it[:, :J // 16],
                        channels=128, num_elems=NLOC, d=1, num_idxs=J)
                    red = sb.tile([128, 96], F32, tag="red")
                    nc.vector.tensor_reduce(
                        red[:, :n],
                        st[:, :J].rearrange("p (n k) -> p n k", k=K),
                        axis=AX.X, op=OP.add)
                    co = CHUNK_OFF[ch]
                    nc.vector.tensor_add(accum[:, co:co + n],
                                         accum[:, co:co + n], red[:, :n])

            # pre-activation: accum * dinv_dst, + bias, relu
            acc2 = pp.tile([128, NGP], F32)
            nc.vector.tensor_tensor(out=acc2[:], in0=accum[:], in1=dinvdst[:],
                                    op=OP.mult)
            nc.scalar.activation(accum[:], acc2[:], ACT.Relu, bias=brepT[:])

            # ---------- pooling: transpose 128-col chunks, one-hot matmul ----------
            pool_psum = pps.tile([HID + 1, NGRAPH], F32)
            nmm = 0
            for ch in range(13):
                tp = ps.tile([128, 128], F32, tag="tps")
                nc.tensor.transpose(tp[:], accum[:, ch * 128:(ch + 1) * 128],
                                    ident[:])
                tpc = sb.tile([128, 128], F32, tag="tpc")
                nc.vector.tensor_copy(tpc[:], tp[:])
                for g in range(8):
                    f17 = sb.tile([128, HID + 1], BF16, tag="f17")
                    nc.vector.tensor_copy(f17[:, :HID], tpc[:, 16 * g:16 * g + 16])
                    nc.vector.tensor_copy(f17[:, HID:HID + 1], ones_t[:])
                    G = sb.tile([128, NGRAPH], BF16, tag="G")
                    nc.vector.tensor_tensor(
                        out=G[:],
                        in0=gidx2[:, ch * 8 + g:ch * 8 + g + 1].to_broadcast([128, NGRAPH]),
                        in1=giota[:], op=OP.is_equal)
                    nc.tensor.matmul(pool_psum[:], f17[:], G[:],
                                     start=(nmm == 0), stop=(nmm == 103))
                    nmm += 1

            # ---------- phase 3: AllReduce pooled sums; replicated head ----------
            pool_s = sb.tile([HID + 1, NGRAPH], F32)
            nc.vector.tensor_copy(pool_s[:], pool_psum[:])
            nc.sync.dma_start(out=poolloc[:], in_=pool_s[:])
            nc.gpsimd.collective_compute(
                "AllReduce", OP.add, replica_groups=groups,
                ins=[poolloc[:]], outs=[poolred[:]])
            pool_r = pp.tile([HID + 1, NGRAPH], F32)
            nc.sync.dma_start(out=pool_r[:], in_=poolred[:])

            # modality MLPs -> [4, 256] tiles in sbuf
            mod_sb = {}
            for m, fdim in MODS:
                mt = pp.tile([fdim if fdim <= 128 else 128,
                              NGRAPH * cdiv(fdim, 128)], F32, tag="mt_" + m)
                if fdim <= 128:
                    nc.sync.dma_start(out=mt[:fdim, :NGRAPH], in_=P[m + 'T'][:])
                else:
                    for k in range(fdim // 128):
                        nc.sync.dma_start(out=mt[:, k * NGRAPH:(k + 1) * NGRAPH],
                                          in_=P[m + 'T'][k * 128:(k + 1) * 128, :])
                wt = pp.tile([fdim if fdim <= 128 else 128,
                              4 * cdiv(fdim, 128)], F32, tag="mw_" + m)
                if fdim <= 128:
                    nc.sync.dma_start(out=wt[:fdim, :4], in_=P[m + 'W'][:])
                else:
                    for k in range(fdim // 128):
                        nc.sync.dma_start(out=wt[:, k * 4:(k + 1) * 4],
                                          in_=P[m + 'W'][k * 128:(k + 1) * 128, :])
                bt = pp.tile([4, 1], F32, tag="mb_" + m)
                nc.sync.dma_start(out=bt[:], in_=P[m + 'b'][:])
                mp = ps.tile([4, NGRAPH], F32, tag="smallps")
                nk = cdiv(fdim, 128)
                for k in range(nk):
                    kk = min(128, fdim - k * 128)
                    nc.tensor.matmul(mp[:], wt[:kk, k * 4:k * 4 + 4],
                                     mt[:kk, k * NGRAPH:(k + 1) * NGRAPH],
                                     start=(k == 0), stop=(k == nk - 1))
                msb = pp.tile([4, NGRAPH], F32, tag="msb_" + m)
                nc.scalar.activation(msb[:], mp[:], ACT.Relu, bias=bt[:])
                mod_sb[m] = msb

            cW1 = pp.tile([32, HID], F32)
            nc.sync.dma_start(out=cW1[:], in_=P['cW1'][:])
            cb1 = pp.tile([HID, 1], F32)
            nc.sync.dma_start(out=cb1[:], in_=P['cb1'][:])
            cW2 = pp.tile([HID, NCLS], F32)
            nc.sync.dma_start(out=cW2[:], in_=P['cW2'][:])
            cb2 = pp.tile([NCLS, 1], F32)
            nc.sync.dma_start(out=cb2[:], in_=P['cb2'][:])

            for gtile in range(2):
                gsl = slice(gtile * 128, (gtile + 1) * 128)
                tp2 = ps.tile([128, HID + 1], F32, tag="smallps")
                nc.tensor.transpose(tp2[:], pool_r[:, gsl], ident[:HID + 1, :HID + 1])
                gsum = sb.tile([128, HID + 1], F32, tag="gsum")
                nc.vector.tensor_copy(gsum[:], tp2[:])
                cnt = sb.tile([128, 1], F32, tag="cnt")
                nc.vector.tensor_scalar_max(cnt[:], gsum[:, HID:HID + 1], 1.0)
                rec = sb.tile([128, 1], F32, tag="rec")
                nc.vector.reciprocal(rec[:], cnt[:])
                comb = sb.tile([128, 32], F32, tag="comb")
                nc.vector.tensor_scalar_mul(comb[:, :HID], gsum[:, :HID], rec[:])
                coff2 = HID
                for m, fdim in MODS:
                    mtp = ps.tile([128, 4], F32, tag="smallps")
                    nc.tensor.transpose(mtp[:], mod_sb[m][:, gsl], ident[:4, :4])
                    nc.vector.tensor_copy(comb[:, coff2:coff2 + 4], mtp[:])
                    coff2 += 4
                ctp = ps.tile([32, 128], F32, tag="smallps")
                nc.tensor.transpose(ctp[:], comb[:], ident[:])
                combT = sb.tile([32, 128], F32, tag="combT")
                nc.vector.tensor_copy(combT[:], ctp[:])
                hp = ps.tile([HID, 128], F32, tag="smallps")
                nc.tensor.matmul(hp[:], cW1[:], combT[:], start=True, stop=True)
                hT = sb.tile([HID, 128], F32, tag="hT")
                nc.scalar.activation(hT[:], hp[:], ACT.Relu, bias=cb1[:])
                lp = ps.tile([NCLS, 128], F32, tag="smallps")
                nc.tensor.matmul(lp[:], cW2[:], hT[:], start=True, stop=True)
                lT = sb.tile([NCLS, 128], F32, tag="lT")
                nc.vector.tensor_scalar_add(lT[:], lp[:], cb2[:])
                ltp = ps.tile([128, NCLS], F32, tag="smallps")
                nc.tensor.transpose(ltp[:], lT[:], ident[:NCLS, :NCLS])
                lg = sb.tile([128, NCLS], F32, tag="lg")
                nc.vector.tensor_copy(lg[:], ltp[:])
                mx = sb.tile([128, 1], F32, tag="mx")
                nc.vector.tensor_reduce(mx[:], lg[:], axis=AX.X, op=OP.max)
                sh = sb.tile([128, NCLS], F32, tag="sh")
                nc.vector.tensor_scalar_sub(sh[:], lg[:], mx[:])
                ex = sb.tile([128, NCLS], F32, tag="ex")
                nc.scalar.activation(ex[:], sh[:], ACT.Exp)
                sm = sb.tile([128, 1], F32, tag="sm")
                nc.vector.tensor_reduce(sm[:], ex[:], axis=AX.X, op=OP.add)
                lns = sb.tile([128, 1], F32, tag="lns")
                nc.scalar.activation(lns[:], sm[:], ACT.Ln)
                fin = sb.tile([128, NCLS], F32, tag="fin")
                nc.vector.tensor_scalar_sub(fin[:], sh[:], lns[:])
                nc.sync.dma_start(out=out[gsl, :], in_=fin[:])
    nc.compile()
    return nc


def _prep(x, edge_index, batch):
    """Host layout: slot permutation + packed per-core gather index streams.

    Returns dict with: pos, node_of, dinv (per slot), gid (per slot),
    Ks [8][NCHUNK], idxall [NCORE, 128, totI] int16, totI.
    """
    x = np.asarray(x, np.float32)
    src = np.asarray(edge_index[0], np.int64)
    dst = np.asarray(edge_index[1], np.int64)
    batch = np.asarray(batch, np.int64)

    deg = np.bincount(dst, minlength=N_NODES).astype(np.int64) + 1

    pos = np.empty(N_NODES, np.int64)
    for c in range(NCORE):
        nodes = np.arange(c * NREAL, (c + 1) * NREAL)
        order = nodes[np.argsort(-deg[nodes], kind='stable')]
        r = np.arange(NREAL)
        pos[order] = c * NLOC + (r % 8) * NGRP + r // 8

    node_of = np.full(NCORE * NLOC, -1, np.int64)
    node_of[pos] = np.arange(N_NODES)
    real = node_of >= 0

    dinv = np.zeros(NCORE * NLOC, np.float32)
    dinv[real] = 1.0 / np.sqrt(deg[node_of[real]].astype(np.float32))
    gid = np.full(NCORE * NLOC, -1.0, np.float32)
    gid[real] = batch[node_of[real]].astype(np.float32)

    psrc = pos[src]
    pdst = pos[dst]
    allslots = pos[np.arange(N_NODES)]
    psrc = np.concatenate([psrc, allslots])
    pdst = np.concatenate([pdst, allslots])

    core_of = pdst // NLOC
    sub = psrc // NLOC
    loc = psrc % NLOC
    dloc = pdst % NLOC
    gg = dloc // NGRP
    jj = dloc % NGRP
    ch_of = np.minimum(jj // 96, 16)
    jloc = jj - np.minimum(ch_of * 96, 1536)

    # counts per (core, sub, group, chunk-local j)
    key = ((core_of * 8 + sub) * NLOC) + dloc
    cnt = np.bincount(key, minlength=NCORE * 8 * NLOC)

    cnt_g = cnt.reshape(NCORE, 8, 8, NGRP)
    Ks = []
    for s in range(NCORE):
        ks = []
        for ch in range(NCHUNK):
            co, n = CHUNK_OFF[ch], CHUNKS[ch]
            ks.append(max(1, int(cnt_g[:, s, :, co:co + n].max())))
        Ks.append(ks)

    order = np.argsort(key, kind='stable')
    ksort = key[order]
    starts = np.searchsorted(ksort, np.arange(NCORE * 8 * NLOC + 1))
    rank = np.empty(len(key), np.int64)
    rank[order] = np.arange(len(key)) - starts[ksort]

    Karr = np.array([[Ks[s][c] for c in range(NCHUNK)] for s in range(NCORE)])
    seg_len = np.array([[CHUNKS[c] * Ks[s][c] for c in range(NCHUNK)]
                        for s in range(NCORE)])  # [sub, chunk] J
    flat = seg_len.flatten()
    seg_off = np.concatenate([[0], np.cumsum(flat)[:-1]]).reshape(NCORE, NCHUNK)
    totJ = int(flat.sum())
    assert totJ % 16 == 0
    totI = totJ // 16

    Kedge = Karr[sub, ch_of]
    assert (rank < Kedge).all(), "rank exceeds K"
    t = jloc * Kedge + rank                      # position within segment/group
    part = 16 * gg + (t % 16)
    free = seg_off[sub, ch_of] // 16 + t // 16

    idxall = np.full((NCORE, 128, totI), ZSLOT, np.int16)
    idxall[core_of, part, free] = loc.astype(np.int16)
    return dict(pos=pos, node_of=node_of, dinv=dinv, gid=gid, Ks=Ks,
                idxall=idxall, totI=totI, seg_off=seg_off, Karr=Karr)


def kernel(x, edge_index, batch, mri, cog, clin, genetic,
           gcn_W, gcn_b, mri_W, mri_b, cog_W, cog_b, clin_W, clin_b,
           gen_W, gen_b, cls_W1, cls_b1, cls_W2, cls_b2):
    meta = _prep(x, edge_index, batch)
    pos, node_of = meta['pos'], meta['node_of']
    dinv, gid, Ks, idxall = meta['dinv'], meta['gid'], meta['Ks'], meta['idxall']

    giota = np.tile(np.arange(NGRAPH, dtype=np.float32), (128, 1))
    brepT = np.asarray(gcn_b, np.float32)[np.arange(128) % HID][:, None].copy()

    in_maps = []
    for c in range(NCORE):
        sl = slice(c * NLOC, (c + 1) * NLOC)
        node_c = node_of[sl]
        realc = node_c >= 0
        xs = np.zeros((NLOC, NODE_DIM), np.float32)
        xs[realc] = (np.asarray(x, np.float32)[node_c[realc]]
                     * dinv[sl][realc, None])
        dinvdst = np.zeros((128, NGP), np.float32)
        gidx2 = np.full((128, 8 * 13), -1.0, np.float32)
        dv = dinv[sl].reshape(8, NGRP)
        gv = gid[sl].reshape(8, NGRP)
        for g in range(8):
            dpad = np.zeros(NGP, np.float32)
            dpad[:NGRP] = dv[g]
            dinvdst[16 * g:16 * g + 16, :] = dpad[None, :]
            gpad = np.full(NGP, -1.0, np.float32)
            gpad[:NGRP] = gv[g]
            gidx2[:, np.arange(13) * 8 + g] = gpad.reshape(13, 128).T
        m = {
            'x16': xs.astype(ml_dtypes.bfloat16),
            'W16': np.asarray(gcn_W, np.float32).astype(ml_dtypes.bfloat16),
            'dinvdst': dinvdst,
            'brepT': brepT,
            'gidx2': gidx2,
            'giota': giota,
            'idxall': idxall[c],
            'cW1': np.asarray(cls_W1, np.float32),
            'cb1': np.asarray(cls_b1, np.float32).reshape(-1, 1),
            'cW2': np.asarray(cls_W2, np.float32),
            'cb2': np.asarray(cls_b2, np.float32).reshape(-1, 1),
        }
        for (mn, fdim), mv, wv, bv in zip(
                MODS, (mri, cog, clin, genetic),
                (mri_W, cog_W, clin_W, gen_W), (mri_b, cog_b, clin_b, gen_b)):
            m[mn + 'T'] = np.asarray(mv, np.float32).T.copy()
            m[mn + 'W'] = np.asarray(wv, np.float32)
            m[mn + 'b'] = np.asarray(bv, np.float32).reshape(-1, 1)
        in_maps.append(m)

    nc = _build(Ks, meta['totI'])
    res = run_bass_kernel_spmd(nc, in_maps, core_ids=list(range(NCORE)))
    return res.results[0]["out"].astype(np.float32)
